# revision 20
# baseline (speedup 1.0000x reference)
"""Trainium2 Bass kernel for nn_AverageAttn (B=4, S=4096, D=H=1024, 8 cores).

out = igate * iQ + fgate * h, where
  avg  = causal cumulative average of iV along seq
  h    = relu(avg @ W1 + b1) @ W2 + b2
  ifg  = sigmoid(concat(iQ, h) @ Wg + bg);  igate, fgate = split(ifg)

Sharding: 8 cores = (batch b, seq half). Each core processes T=2048 tokens.

v3 design:
 - Host supplies all activations pre-transposed (T-orientation
   [feature, token]) in bf16; output leaves T-oriented bf16.
 - Cumsum on Vector/Pool engines (tensor_tensor_scan); first-half carry
   via Activation-engine accumulate; avg = (scan + carry) * (1/n) in one
   scalar_tensor_tensor.  Zero PE scan work.
 - FFN1/FFN2 and the h-half of the gate in fp8 e4m3 DoubleRow (K=256 per
   instruction); gate iQ-half bf16.  PSUM accumulates fp32.
 - Startup latency hidden: the gate iQ-half for gate chunks 0..7 runs
   FIRST (needs only qT), staged to SBUF as bf16 zq; later only the
   h-half is accumulated and added back before the sigmoid.
 - One [128,2048] activation per output chunk over a 4-bank PSUM tile.
"""

import numpy as np

B, S, D = 4, 4096, 1024
H = 1024
T = S // 2              # tokens per core
P = 128
ND = D // P             # 8 feature chunks
NG = 2 * D // P         # 16 gate chunks
NP = ND // 2            # 4 DoubleRow K-pairs
NBLK = 4                # 512-token matmul blocks
BT = T // NBLK          # 512
POOL_SCAN_D = (1, 4, 7)  # scan chunks routed to the Pool engine


def _build_program():
    import contextlib
    import concourse.bass as bass  # noqa: F401
    import concourse.tile as tile
    from concourse import mybir, bacc

    f32 = mybir.dt.float32
    bf16 = mybir.dt.bfloat16
    fp8 = mybir.dt.float8e4
    Relu = mybir.ActivationFunctionType.Relu
    Ident = mybir.ActivationFunctionType.Identity
    Sigm = mybir.ActivationFunctionType.Sigmoid
    DR = mybir.MatmulPerfMode.DoubleRow
    Add = mybir.AluOpType.add
    Mult = mybir.AluOpType.mult
    Bypass = mybir.AluOpType.bypass

    nc = bacc.Bacc("TRN2", target_bir_lowering=False)

    vTd = nc.dram_tensor("vTd", [ND, P, T], bf16, kind="ExternalInput")
    vpreT = nc.dram_tensor("vpreT", [ND, P, T], fp8, kind="ExternalInput")
    qTd = nc.dram_tensor("qTd", [ND, P, T], bf16, kind="ExternalInput")
    denr = nc.dram_tensor("denr", [P, T], bf16, kind="ExternalInput")
    w1s = nc.dram_tensor("w1s", [ND, P, NP, 2, P], fp8, kind="ExternalInput")
    w2s = nc.dram_tensor("w2s", [ND, P, NP, 2, P], fp8, kind="ExternalInput")
    wgt = nc.dram_tensor("wgt", [ND, P, ND, P], bf16, kind="ExternalInput")
    wgtb = nc.dram_tensor("wgtb", [ND, P, NP, 2, P], fp8,
                          kind="ExternalInput")
    qp8d = nc.dram_tensor("qp8d", [NP, P, 2, T], fp8, kind="ExternalInput")
    wgb = nc.dram_tensor("wgb", [NG, P, NP, 2, P], fp8, kind="ExternalInput")
    bpk = nc.dram_tensor("bpk", [P, 2 * ND + NG], f32,
                         kind="ExternalInput")
    o = nc.dram_tensor("o", [ND, P, T], bf16, kind="ExternalOutput")

    with tile.TileContext(nc) as tc:
        ctx = contextlib.ExitStack()
        with ctx:
            cpool = ctx.enter_context(tc.tile_pool(name="consts", bufs=1))
            qpool = ctx.enter_context(tc.tile_pool(name="qq", bufs=ND))
            zqpool = ctx.enter_context(tc.tile_pool(name="zq", bufs=ND))
            vpool = ctx.enter_context(tc.tile_pool(name="vstream", bufs=3))
            prepool = ctx.enter_context(tc.tile_pool(name="prestream", bufs=2))
            scanpool = ctx.enter_context(tc.tile_pool(name="scans", bufs=2))
            scrpool = ctx.enter_context(tc.tile_pool(name="scratch", bufs=1))
            vsumpool = ctx.enter_context(tc.tile_pool(name="vsum", bufs=2))
            avgpool = ctx.enter_context(tc.tile_pool(name="avg", bufs=NP))
            h1pool = ctx.enter_context(tc.tile_pool(name="h1", bufs=NP))
            hpool = ctx.enter_context(tc.tile_pool(name="hh", bufs=NP))
            wpool = ctx.enter_context(tc.tile_pool(name="w12", bufs=2))
            gwpool = ctx.enter_context(tc.tile_pool(name="gw", bufs=2))
            gatepool = ctx.enter_context(tc.tile_pool(name="gates", bufs=3))
            ewpool = ctx.enter_context(tc.tile_pool(name="ew", bufs=1))
            opool = ctx.enter_context(tc.tile_pool(name="outs", bufs=1))
            pspool = ctx.enter_context(
                tc.tile_pool(name="psmm", bufs=2, space="PSUM"))

            # ---- qT chunks + vT interleaved on the SP queue ---------------
            # qc0-3 first (earliest PE dep), then alternate vt/qc so the
            # DVE scan chain starts early too.  vpre on Pool SWDGE.
            qTc = [qpool.tile([P, T], bf16, tag="qT", name="qt")
                   for _ in range(ND)]
            vts = [vpool.tile([P, T], bf16, tag="v", name="vt")
                   for _ in range(ND)]
            pres = []
            # first two gate-weight slabs ride the fast SP queue
            tops_early = {}
            for g in (0, 1):
                t_ = gwpool.tile([P, ND, P], bf16, tag="wgt", name="top")
                nc.sync.dma_start(t_[:], wgt[g])
                tops_early[g] = t_
            # front-load qT (PE fill) while spreading vT (DVE scan chain)
            sp_order = [('q', 0), ('q', 1), ('v', 0), ('q', 2), ('q', 3),
                        ('v', 1), ('q', 4), ('q', 5), ('q', 6), ('q', 7),
                        ('v', 2), ('v', 3), ('v', 4), ('v', 5), ('v', 6),
                        ('v', 7)]
            for kind, i in sp_order:
                if kind == 'q':
                    nc.sync.dma_start(qTc[i][:], qTd[i])
                else:
                    nc.sync.dma_start(vts[i][:], vTd[i])
            for d in range(ND):
                pre = prepool.tile([P, T], fp8, tag="pre", name="pre")
                nc.gpsimd.dma_start(pre[:], vpreT[d])
                pres.append(pre)

            qp8 = [qpool.tile([P, 2, T], fp8, tag="qp8", name="qp8")
                   for _ in range(NP)]
            for p in range(NP):
                nc.sync.dma_start(qp8[p][:], qp8d[p])

            def qT(c):  # [P, T] view of iQ chunk c
                return qTc[c][:]

            # denr after the first couple of gate-weight slabs (Act queue)
            denrT = cpool.tile([P, T], bf16, tag="denr")
            bT = cpool.tile([P, 2 * ND + NG], f32, tag="bpk")

            # ---- EARLY: gate iQ-half for g=0..7, staged to SBUF bf16 ------
            zq = []
            for g in range(ND):
                if g in tops_early:
                    top = tops_early[g]
                else:
                    top = gwpool.tile([P, ND, P], bf16, tag="wgt", name="top")
                    nc.scalar.dma_start(top[:], wgt[g])
                if g == 2:
                    nc.scalar.dma_start(denrT[:], denr[:])
                if g == ND - 1:
                    nc.scalar.dma_start(bT[:], bpk[:])
                ps = pspool.tile([P, NBLK * BT], f32, tag="mm", name="ps")
                for c in range(ND):
                    for blk in range(NBLK):
                        nc.tensor.matmul(
                            ps[:, blk * BT:(blk + 1) * BT], top[:, c, :],
                            qT(c)[:, blk * BT:(blk + 1) * BT],
                            start=(c == 0), stop=(c == ND - 1))
                zt = zqpool.tile([P, T], bf16, tag="zq", name="zq")
                nc.scalar.activation(zt[:], ps[:], Ident)
                zq.append(zt)

            # ---- scan phase: avg8 pair tiles [P, 2, T] fp8 ----------------
            # cumsum with the first-half carry as the scan initial (DVE);
            # the * (1/n) multiply runs on Pool (plain TensorTensor).
            avg8 = [avgpool.tile([P, 2, T], fp8, tag="avg", name="avg8")
                    for _ in range(NP)]
            vsums = [vsumpool.tile([P, 4], f32, tag="vsum", name="vsum")
                     for _ in range(2)]
            for d in range(ND):
                vsum = vsums[d // 4][:, d % 4:d % 4 + 1]
                scr = scrpool.tile([P, T], fp8, tag="scr", name="scr")
                nc.scalar.activation(scr[:], pres[d][:], Ident,
                                     accum_out=vsum)
                sc = scanpool.tile([P, T], bf16, tag="scan", name="sc")
                nc.vector.tensor_tensor_scan(sc[:], vts[d][:], vts[d][:],
                                             vsum, Add, Bypass)
                nc.vector.tensor_mul(avg8[d // 2][:, d % 2, :], sc[:],
                                     denrT[:])

            # ---- FFN1: h1 = relu(avg @ W1 + b1), fp8 DoubleRow ------------
            h1 = [h1pool.tile([P, 2, T], fp8, tag="h1", name="h1")
                  for _ in range(NP)]
            for j in range(ND):
                w1t = wpool.tile([P, NP, 2, P], fp8, tag="w12", name="w1t")
                nc.scalar.dma_start(w1t[:], w1s[j])
                ps = pspool.tile([P, NBLK * BT], f32, tag="mm", name="ps")
                for p in range(NP):
                    for blk in range(NBLK):
                        nc.tensor.matmul(
                            ps[:, blk * BT:(blk + 1) * BT], w1t[:, p, :, :],
                            avg8[p][:, :, blk * BT:(blk + 1) * BT],
                            start=(p == 0), stop=(p == NP - 1),
                            perf_mode=DR)
                nc.scalar.activation(h1[j // 2][:, j % 2, :], ps[:], Relu,
                                     bias=bT[:, j:j + 1])

            # ---- FFN2: h = h1 @ W2 + b2, fp8 DoubleRow --------------------
            hh = [hpool.tile([P, 2, T], fp8, tag="hh", name="hh")
                  for _ in range(NP)]
            for j in range(ND):
                w2t = wpool.tile([P, NP, 2, P], fp8, tag="w12", name="w2t")
                nc.scalar.dma_start(w2t[:], w2s[j])
                ps = pspool.tile([P, NBLK * BT], f32, tag="mm", name="ps")
                for p in range(NP):
                    for blk in range(NBLK):
                        nc.tensor.matmul(
                            ps[:, blk * BT:(blk + 1) * BT], w2t[:, p, :, :],
                            h1[p][:, :, blk * BT:(blk + 1) * BT],
                            start=(p == 0), stop=(p == NP - 1),
                            perf_mode=DR)
                nc.scalar.activation(hh[j // 2][:, j % 2, :], ps[:], Ident,
                                     bias=bT[:, ND + j:ND + j + 1])

            # ---- gate rest + output ---------------------------------------
            def h_half(ps, g, start):
                bot = gwpool.tile([P, NP, 2, P], fp8, tag="wgb", name="bot")
                nc.scalar.dma_start(bot[:], wgb[g])
                for p in range(NP):
                    for blk in range(NBLK):
                        nc.tensor.matmul(
                            ps[:, blk * BT:(blk + 1) * BT], bot[:, p, :, :],
                            hh[p][:, :, blk * BT:(blk + 1) * BT],
                            start=(start and p == 0), stop=(p == NP - 1),
                            perf_mode=DR)

            def fg_mms(gp):
                g = gp + ND
                topb = gwpool.tile([P, NP, 2, P], fp8, tag="wgtb", name="topb")
                nc.scalar.dma_start(topb[:], wgtb[gp])
                ps2 = pspool.tile([P, NBLK * BT], f32, tag="mm", name="ps2")
                for p in range(NP):
                    for blk in range(NBLK):
                        nc.tensor.matmul(
                            ps2[:, blk * BT:(blk + 1) * BT], topb[:, p, :, :],
                            qp8[p][:, :, blk * BT:(blk + 1) * BT],
                            start=(p == 0), stop=False, perf_mode=DR)
                h_half(ps2, g, start=False)
                return ps2

            for gp in range(ND):
                # igate chunk gp: h-half into PSUM, add staged zq, sigmoid
                last = gp == ND - 1
                nb = NBLK if last else 1
                bw = T // nb
                ps = pspool.tile([P, NBLK * BT], f32, tag="mm", name="ps")
                ps2 = None
                if last:
                    ps2 = fg_mms(gp)
                h_half(ps, gp, start=True)
                ig = gatepool.tile([P, T], bf16, tag="gate", name="ig")
                for x in range(nb):
                    sl = slice(x * bw, (x + 1) * bw)
                    nc.vector.tensor_add(zq[gp][:, sl], ps[:, sl],
                                         zq[gp][:, sl])
                    nc.scalar.activation(ig[:, sl], zq[gp][:, sl], Sigm,
                                         bias=bT[:, 2 * ND + gp:2 * ND + gp + 1])

                # fgate chunk gp+8: full accumulation
                g = gp + ND
                if ps2 is None:
                    ps2 = fg_mms(gp)
                fg = gatepool.tile([P, T], bf16, tag="gate", name="fg")
                tmp = ewpool.tile([P, T], bf16, tag="tmp", name="tmp")
                ob = opool.tile([P, T], bf16, tag="ob", name="ob")
                eng = nc.vector if last else nc.gpsimd
                for x in range(nb):
                    sl = slice(x * bw, (x + 1) * bw)
                    nc.scalar.activation(fg[:, sl], ps2[:, sl], Sigm,
                                         bias=bT[:, 2 * ND + g:2 * ND + g + 1])
                    nc.vector.tensor_mul(tmp[:, sl], ig[:, sl], qT(gp)[:, sl])
                    eng.tensor_mul(ob[:, sl], fg[:, sl],
                                   hh[gp // 2][:, gp % 2, sl])
                    nc.vector.tensor_add(ob[:, sl], ob[:, sl], tmp[:, sl])
                    nc.sync.dma_start(o[gp, :, sl], ob[:, sl])

    nc.finalize()
    return nc


_CACHED = {}
_last_result = None


def kernel(iQ, iV, W1, b1, W2, b2, Wg, bg):
    import sys
    if '/opt/trn_rl_repo' not in sys.path:
        sys.path.insert(0, '/opt/trn_rl_repo')
    import ml_dtypes
    from concourse.bass_utils import run_bass_kernel_spmd

    BF = ml_dtypes.bfloat16
    F8 = ml_dtypes.float8_e4m3

    iQ = np.asarray(iQ, np.float32)
    iV = np.asarray(iV, np.float32)
    W1 = np.asarray(W1, np.float32)
    b1 = np.asarray(b1, np.float32)
    W2 = np.asarray(W2, np.float32)
    b2 = np.asarray(b2, np.float32)
    Wg = np.asarray(Wg, np.float32)
    bg = np.asarray(bg, np.float32)

    if 'nc' not in _CACHED:
        _CACHED['nc'] = _build_program()
    nc = _CACHED['nc']

    # weight slabs, lhsT layouts (see _build_program dram shapes)
    def dr_slab(W, n_out):
        # [j, k, p, i, m] with K index (p*2+i)*128+k
        return np.ascontiguousarray(
            W.reshape(NP, 2, P, n_out, P).transpose(3, 2, 0, 1, 4)).astype(F8)

    w1s = dr_slab(W1, ND)
    w2s = dr_slab(W2, ND)
    wgt = np.ascontiguousarray(
        Wg[:D, :D].reshape(ND, P, ND, P).transpose(2, 1, 0, 3)).astype(BF)
    wgtb = dr_slab(Wg[:D, D:], ND)
    wgb = dr_slab(Wg[D:], NG)
    bpk = np.ascontiguousarray(np.concatenate([
        b1.reshape(ND, P).T, b2.reshape(ND, P).T, bg.reshape(NG, P).T],
        axis=1))
    zpre = np.zeros((ND, P, T), F8)

    def t_orient(x):  # [T, D] f32 -> [ND, P, T] bf16
        return np.ascontiguousarray(x.T.reshape(ND, P, T)).astype(BF)

    in_maps = []
    for core in range(8):
        b, half = core // 2, core % 2
        off = half * T
        den = np.ascontiguousarray(np.broadcast_to(
            1.0 / np.arange(off + 1, off + T + 1, dtype=np.float32),
            (P, T))).astype(BF)
        in_maps.append({
            "qTd": t_orient(iQ[b, off:off + T]),
            "vTd": t_orient(iV[b, off:off + T]),
            "vpreT": (np.ascontiguousarray(
                iV[b, :T].T.reshape(ND, P, T)).astype(F8)
                if half == 1 else zpre),
            "denr": den,
            "w1s": w1s, "w2s": w2s, "wgt": wgt, "wgb": wgb,
            "wgtb": wgtb,
            "qp8d": np.ascontiguousarray(
                iQ[b, off:off + T].T.reshape(NP, 2, P, T)
                .transpose(0, 2, 1, 3)).astype(F8),
            "bpk": bpk,
        })

    res = run_bass_kernel_spmd(nc, in_maps, core_ids=list(range(8)))
    global _last_result
    _last_result = res

    out = np.empty((B, S, D), np.float32)
    for core in range(8):
        b, half = core // 2, core % 2
        ot = np.asarray(res.results[core]["o"], dtype=np.float32)
        out[b, half * T:(half + 1) * T] = \
            ot.transpose(2, 0, 1).reshape(T, D)
    return out
